# revision 3
# baseline (speedup 1.0000x reference)
"""Trainium2 Bass kernel for nn_Light_Spattention (linearized attention / GNN
message passing).

Math (per (b,t) slice, x: [N, F], N=2048 nodes, F=256 features, 4 heads x 64):
    q = x @ Q ; k = x @ K ; xh = x
    summary_h = k_h^T @ x_h                       (contract nodes)
    attn_h    = q_h @ summary_h / N
    out       = sig(alpha_h) * x + sig(beta_h) * attn_h

Refactored via the Gram matrix to eliminate the N-sized projections:
    G   = x^T x                                   [256, 256]
    P   = K^T G                                   [256, 256]
    Sig_h = P[h-block diag]                       4 x [64, 64]
    W   = Qs^T_block . Sig_bd + diag(sig(alpha))  [256, 256]
          (Qs = Q scaled per-head-column by sig(beta)/N)
    out = x @ W

Per-core work = 6 of the 48 (b,t) slices (pure data parallel, no collectives).
All matmuls run as float32r (TF32-class single-pass PE mode, fp32 accumulate).
x^T tiles (needed as matmul lhsT for out = x @ W) are produced on-chip with
PE transpose-mode, interleaved with the Gram accumulation.
"""

import numpy as np

import concourse.bass as bass  # noqa: F401  (bass types reachable via tile/bacc)
import concourse.tile as tile
from concourse import bacc, mybir
from concourse.bass_utils import run_bass_kernel_spmd

B, T, NN, DIM, HEAD = 4, 12, 2048, 256, 4
HD = DIM // HEAD            # 64
BT = B * T                  # 48
N_CORES = 8
BT_PER_CORE = BT // N_CORES  # 6
NT = NN // 128              # 16 node tiles per slice
EC = DIM // 128             # 2 feature chunks of 128

F32 = mybir.dt.float32
F32R = mybir.dt.float32r

# How many of the 16 per-slice transpose pairs are emitted interleaved with the
# Gram matmuls; the rest are emitted between the small-matmul stages to keep PE
# busy while DVE does the PSUM->SBUF handoffs.
TP_EARLY = 10


def build_nc():
    nc = bacc.Bacc(None, target_bir_lowering=False)

    x_d = nc.dram_tensor("x", [BT_PER_CORE, NN, DIM], F32R, kind="ExternalInput")
    kw_d = nc.dram_tensor("kw", [DIM, DIM], F32R, kind="ExternalInput")
    qst_d = nc.dram_tensor("qst", [DIM, DIM], F32R, kind="ExternalInput")
    dmat_d = nc.dram_tensor("dmat", [DIM, DIM], F32, kind="ExternalInput")
    ident_d = nc.dram_tensor("ident", [128, 128], F32R, kind="ExternalInput")
    zed_d = nc.dram_tensor("zed", [128, EC * DIM], F32R, kind="ExternalInput")
    out_d = nc.dram_tensor("out", [BT_PER_CORE, NN, DIM], F32, kind="ExternalOutput")

    with tile.TileContext(nc) as tc:
        with (
            tc.tile_pool(name="consts", bufs=1) as consts,
            tc.tile_pool(name="xin", bufs=2) as xin,
            tc.tile_pool(name="xtp", bufs=2) as xtp,
            tc.tile_pool(name="outp", bufs=2) as outp,
            tc.tile_pool(name="small", bufs=2) as small,
            tc.tile_pool(name="ps_g", bufs=2, space="PSUM") as ps_g,
            tc.tile_pool(name="ps_t", bufs=2, space="PSUM") as ps_t,
            tc.tile_pool(name="ps_s", bufs=2, space="PSUM") as ps_s,
            tc.tile_pool(name="ps_o", bufs=2, space="PSUM") as ps_o,
        ):
            # --- constants, loaded once ---
            kw = consts.tile([128, EC, DIM], F32R)
            nc.sync.dma_start(out=kw, in_=kw_d.rearrange("(c p) j -> p c j", p=128))
            qst = consts.tile([128, EC, DIM], F32R)
            nc.sync.dma_start(out=qst, in_=qst_d.rearrange("(c p) e -> p c e", p=128))
            dmat = consts.tile([128, EC, DIM], F32)
            nc.sync.dma_start(out=dmat, in_=dmat_d.rearrange("(c p) e -> p c e", p=128))
            ident = consts.tile([128, 128], F32R)
            nc.sync.dma_start(out=ident, in_=ident_d[:, :])
            # block-diagonal summary holder; off-diagonal blocks stay zero
            sbd = consts.tile([128, EC, DIM], F32R)
            nc.sync.dma_start(out=sbd, in_=zed_d.rearrange("p (c f) -> p c f", c=EC))

            for i in range(BT_PER_CORE):
                # node n = p*NT + t lives at x_sb[p, t, :]; 16KiB contiguous per
                # partition in HBM -> full-rate DMA.
                x_sb = xin.tile([128, NT, DIM], F32R, tag="x")
                nc.sync.dma_start(
                    out=x_sb, in_=x_d[i].rearrange("(p t) d -> p t d", p=128)
                )

                xt_sb = xtp.tile([128, EC, NN], F32R, tag="xt")
                out_sb = outp.tile([128, NT, DIM], F32, tag="o")

                def emit_transpose(t):
                    for ecc in range(EC):
                        tp = ps_t.tile([128, 128], F32R, tag="tp")
                        nc.tensor.transpose(
                            tp, x_sb[:, t, ecc * 128 : (ecc + 1) * 128], ident
                        )
                        nc.vector.tensor_copy(
                            out=xt_sb[:, ecc, t * 128 : (t + 1) * 128], in_=tp
                        )

                # --- G = x^T x (PSUM-accumulate over the 16 node tiles),
                #     transposes interleaved so PE stays dense ---
                g_ps = [ps_g.tile([128, DIM], F32, tag="g", name=f"g{i}_{c}") for c in range(EC)]
                for t in range(NT):
                    for ecc in range(EC):
                        nc.tensor.matmul(
                            g_ps[ecc],
                            x_sb[:, t, ecc * 128 : (ecc + 1) * 128],
                            x_sb[:, t, :],
                            start=(t == 0),
                            stop=(t == NT - 1),
                        )
                    if t < TP_EARLY:
                        emit_transpose(t)

                g_sb = small.tile([128, EC, DIM], F32R, tag="g_sb")
                for ecc in range(EC):
                    nc.vector.tensor_copy(out=g_sb[:, ecc, :], in_=g_ps[ecc])

                emit_transpose(TP_EARLY)
                emit_transpose(TP_EARLY + 1)

                # --- P = K^T G ---
                p_ps = [ps_s.tile([128, DIM], F32, tag="pw", name=f"p{i}_{c}") for c in range(EC)]
                for jc in range(EC):
                    for ecc in range(EC):
                        nc.tensor.matmul(
                            p_ps[jc],
                            kw[:, ecc, jc * 128 : (jc + 1) * 128],
                            g_sb[:, ecc, :],
                            start=(ecc == 0),
                            stop=(ecc == EC - 1),
                        )

                emit_transpose(TP_EARLY + 2)
                emit_transpose(TP_EARLY + 3)

                # --- extract diagonal head blocks into the block-diag holder ---
                for h in range(HEAD):
                    jc, r = divmod(h, 2)
                    r0 = r * HD
                    nc.vector.tensor_copy(
                        out=sbd[r0 : r0 + HD, jc, h * HD : (h + 1) * HD],
                        in_=p_ps[jc][r0 : r0 + HD, h * HD : (h + 1) * HD],
                    )

                # --- W = Qs^T . Sig_bd (+ diag(sig(alpha)) via DVE add) ---
                w_ps = [ps_s.tile([128, DIM], F32, tag="pw", name=f"w{i}_{c}") for c in range(EC)]
                for ecc in range(EC):
                    for sc in range(EC):
                        nc.tensor.matmul(
                            w_ps[ecc],
                            qst[:, sc, ecc * 128 : (ecc + 1) * 128],
                            sbd[:, sc, :],
                            start=(sc == 0),
                            stop=(sc == EC - 1),
                        )

                emit_transpose(TP_EARLY + 4)
                emit_transpose(TP_EARLY + 5)

                w_sb = small.tile([128, EC, DIM], F32R, tag="w_sb")
                for ecc in range(EC):
                    nc.vector.tensor_add(
                        out=w_sb[:, ecc, :], in0=w_ps[ecc], in1=dmat[:, ecc, :]
                    )

                # --- out = x @ W  (lhsT = x^T tiles) ---
                for t in range(NT):
                    o_ps = ps_o.tile([128, DIM], F32, tag="att")
                    for ecc in range(EC):
                        nc.tensor.matmul(
                            o_ps,
                            xt_sb[:, ecc, t * 128 : (t + 1) * 128],
                            w_sb[:, ecc, :],
                            start=(ecc == 0),
                            stop=(ecc == EC - 1),
                        )
                    nc.scalar.copy(out=out_sb[:, t, :], in_=o_ps)

                nc.gpsimd.dma_start(
                    out=out_d[i].rearrange("(p t) d -> p t d", p=128), in_=out_sb
                )

    nc.finalize()
    return nc


def _host_prep(x, Q, K, alpha, beta):
    x = np.ascontiguousarray(np.asarray(x, dtype=np.float32))
    Q = np.asarray(Q, dtype=np.float32)
    K = np.asarray(K, dtype=np.float32)
    sa = (1.0 / (1.0 + np.exp(-np.asarray(alpha, dtype=np.float32)))).reshape(HEAD)
    sb = (1.0 / (1.0 + np.exp(-np.asarray(beta, dtype=np.float32)))).reshape(HEAD)

    scale_cols = np.repeat(sb / NN, HD).astype(np.float32)        # [256]
    qs = (Q * scale_cols[None, :]).astype(np.float32)
    qst = np.ascontiguousarray(qs.T)
    kw = np.ascontiguousarray(K)
    sa_cols = np.repeat(sa, HD).astype(np.float32)
    dmat = np.ascontiguousarray(np.diag(sa_cols).astype(np.float32))
    ident = np.ascontiguousarray(np.eye(128, dtype=np.float32))
    zed = np.zeros((128, EC * DIM), dtype=np.float32)

    x48 = x.reshape(BT, NN, DIM)
    in_maps = []
    for c in range(N_CORES):
        shard = np.ascontiguousarray(x48[c * BT_PER_CORE : (c + 1) * BT_PER_CORE])
        in_maps.append(
            {
                "x": shard,
                "kw": kw,
                "qst": qst,
                "dmat": dmat,
                "ident": ident,
                "zed": zed,
            }
        )
    return in_maps


def run(x, Q, K, alpha, beta, **spmd_kwargs):
    """Build, run on 8 cores, gather. Returns (out, BassKernelResults, nc)."""
    in_maps = _host_prep(x, Q, K, alpha, beta)
    nc = build_nc()
    res = run_bass_kernel_spmd(nc, in_maps, core_ids=list(range(N_CORES)), **spmd_kwargs)
    out48 = np.concatenate([res.results[c]["out"] for c in range(N_CORES)], axis=0)
    out = out48.reshape(B, T, NN, DIM).astype(np.float32, copy=False)
    return out, res, nc


def kernel(x, Q, K, alpha, beta):
    out, _, _ = run(x, Q, K, alpha, beta)
    return out


# revision 6
# speedup vs baseline: 1.1276x; 1.1276x over previous
"""Trainium2 Bass kernel for nn_Light_Spattention (linearized attention / GNN
message passing).

Math (per (b,t) slice, x: [N, F], N=2048 nodes, F=256 features, 4 heads x 64):
    q = x @ Q ; k = x @ K ; xh = x
    summary_h = k_h^T @ x_h                       (contract nodes)
    attn_h    = q_h @ summary_h / N
    out       = sig(alpha_h) * x + sig(beta_h) * attn_h

Refactored via the Gram matrix to eliminate the N-sized projections:
    G   = x^T x                                   [256, 256]
    P   = K^T G                                   [256, 256]
    Sig_h = P[h-block diag]                       4 x [64, 64]
    W   = Qs^T . Sig_bd + diag(sig(alpha))        [256, 256]
          (Qs = Q scaled per-head-column by sig(beta)/N)
    out = x @ W

Per-core work = 6 of the 48 (b,t) slices (pure data parallel, no collectives).
All matmuls run as float32r (TF32-class single-pass PE mode, fp32 accumulate).
x^T tiles (matmul lhsT for out = x @ W) are made with PE transpose-mode; four
128x128 transposes share one PSUM bank so a single DVE/ACT op evicts them.
PSUM->SBUF eviction work is split across DVE and ACT to balance engine load.
"""

import numpy as np

import concourse.bass as bass  # noqa: F401
import concourse.tile as tile
from concourse import bacc, mybir
from concourse.bass_utils import run_bass_kernel_spmd

B, T, NN, DIM, HEAD = 4, 12, 2048, 256, 4
HD = DIM // HEAD            # 64
BT = B * T                  # 48
N_CORES = 8
BT_PER_CORE = BT // N_CORES  # 6
NT = NN // 128              # 16 node tiles per slice
HT = NT // 2                # 8 node tiles per half
EC = DIM // 128             # 2 feature chunks of 128

F32 = mybir.dt.float32
F32R = mybir.dt.float32r


def build_nc():
    nc = bacc.Bacc(None, target_bir_lowering=False)

    x_d = nc.dram_tensor("x", [BT_PER_CORE, NN, DIM], F32R, kind="ExternalInput")
    kw_d = nc.dram_tensor("kw", [DIM, DIM], F32R, kind="ExternalInput")
    qst_d = nc.dram_tensor("qst", [DIM, DIM], F32R, kind="ExternalInput")
    dmat_d = nc.dram_tensor("dmat", [DIM, DIM], F32, kind="ExternalInput")
    ident_d = nc.dram_tensor("ident", [128, 128], F32R, kind="ExternalInput")
    zed_d = nc.dram_tensor("zed", [128, EC * DIM], F32R, kind="ExternalInput")
    out_d = nc.dram_tensor("out", [BT_PER_CORE, NN, DIM], F32, kind="ExternalOutput")

    with tile.TileContext(nc) as tc:
        with (
            tc.tile_pool(name="consts", bufs=1) as consts,
            tc.tile_pool(name="xin", bufs=4) as xin,
            tc.tile_pool(name="xtp", bufs=2) as xtp,
            tc.tile_pool(name="outp", bufs=4) as outp,
            tc.tile_pool(name="small", bufs=2) as small,
            # 2-bank tile for the two interleaved Gram accumulation groups
            tc.tile_pool(name="ps_g", bufs=1, space="PSUM") as ps_g,
            # shared one-bank scratch: transpose quads, P/W, attn pairs
            tc.tile_pool(name="ps_b", bufs=4, space="PSUM") as ps_b,
        ):
            # --- constants, loaded once ---
            kw = consts.tile([128, EC, DIM], F32R)
            nc.sync.dma_start(out=kw, in_=kw_d.rearrange("(c p) j -> p c j", p=128))
            qst = consts.tile([128, EC, DIM], F32R)
            nc.sync.dma_start(out=qst, in_=qst_d.rearrange("(c p) e -> p c e", p=128))
            dmat = consts.tile([128, EC, DIM], F32)
            nc.sync.dma_start(
                out=dmat, in_=dmat_d.rearrange("(c p) e -> p c e", p=128)
            )
            ident = consts.tile([128, 128], F32R)
            nc.sync.dma_start(out=ident, in_=ident_d[:, :])
            # block-diagonal summary holder; off-diagonal blocks stay zero
            sbd = consts.tile([128, EC, DIM], F32R)
            nc.sync.dma_start(out=sbd, in_=zed_d.rearrange("p (c f) -> p c f", c=EC))

            for i in range(BT_PER_CORE):
                # node n = p*NT + t lives at x_half[p, t, :]; 8KiB contiguous
                # per partition per half -> full-rate DMA, and the second half
                # can land while compute on the first is underway.
                x_hbm = x_d[i].rearrange("(p t) d -> p t d", p=128)
                x_half = []
                for hh in range(2):
                    xh_t = xin.tile([128, HT, DIM], F32R, tag="x", name=f"x{i}_{hh}")
                    nc.sync.dma_start(
                        out=xh_t, in_=x_hbm[:, hh * HT : (hh + 1) * HT, :]
                    )
                    x_half.append(xh_t)

                def xs(t):
                    return x_half[t // HT][:, t % HT, :]

                xt_sb = xtp.tile([128, EC, NN], F32R, tag="xt")

                # --- G = x^T x (two PSUM-accumulate groups over 16 node tiles)
                #     with transposes interleaved; 4 transposes share one bank
                #     and are evicted with a single strided copy ---
                g_ps = ps_g.tile([128, EC, 512], F32, tag="g")
                for tq in range(HT):
                    t0 = 2 * tq
                    for t in (t0, t0 + 1):
                        for ecc in range(EC):
                            nc.tensor.matmul(
                                g_ps[:, ecc, 0:DIM],
                                xs(t)[:, ecc * 128 : (ecc + 1) * 128],
                                xs(t),
                                start=(t == 0),
                                stop=(t == NT - 1),
                            )
                    quad = ps_b.tile([128, 4, 128], F32R, tag="bank", name=f"q{i}_{tq}")
                    # slot order: [t0|e0, t1|e0, t0|e1, t1|e1] so the flat bank
                    # maps onto xt_sb[:, :, t0*128 : t0*128+256]
                    for ecc in range(EC):
                        for j, t in enumerate((t0, t0 + 1)):
                            nc.tensor.transpose(
                                quad[:, ecc * 2 + j, :],
                                xs(t)[:, ecc * 128 : (ecc + 1) * 128],
                                ident,
                            )
                    src = quad.rearrange("p (a b) c -> p a (b c)", a=EC)
                    dst = xt_sb[:, :, t0 * 128 : t0 * 128 + 256]
                    if tq % 2 == 0:
                        nc.vector.tensor_copy(out=dst, in_=src)
                    else:
                        nc.scalar.copy(out=dst, in_=src)

                g_sb = small.tile([128, EC, DIM], F32R, tag="g_sb")
                nc.scalar.copy(out=g_sb, in_=g_ps[:, :, 0:DIM])

                # --- P = K^T G (two sequential groups in one bank) ---
                p_ps = ps_b.tile([128, 2, DIM], F32, tag="bank", name=f"p{i}")
                for jc in range(EC):
                    for ecc in range(EC):
                        nc.tensor.matmul(
                            p_ps[:, jc, :],
                            kw[:, ecc, jc * 128 : (jc + 1) * 128],
                            g_sb[:, ecc, :],
                            start=(ecc == 0),
                            stop=(ecc == EC - 1),
                        )

                # --- extract diagonal head blocks into the block-diag holder ---
                for h in range(HEAD):
                    jc, r = divmod(h, 2)
                    r0 = r * HD
                    src = p_ps[r0 : r0 + HD, jc, h * HD : (h + 1) * HD]
                    dst = sbd[r0 : r0 + HD, jc, h * HD : (h + 1) * HD]
                    if h % 2 == 0:
                        nc.vector.tensor_copy(out=dst, in_=src)
                    else:
                        nc.scalar.copy(out=dst, in_=src)

                # --- W = Qs^T . Sig_bd, then += diag(sig(alpha)) ---
                w_ps = ps_b.tile([128, 2, DIM], F32, tag="bank", name=f"w{i}")
                for ecc in range(EC):
                    for sc in range(EC):
                        nc.tensor.matmul(
                            w_ps[:, ecc, :],
                            qst[:, sc, ecc * 128 : (ecc + 1) * 128],
                            sbd[:, sc, :],
                            start=(sc == 0),
                            stop=(sc == EC - 1),
                        )
                w_sb = small.tile([128, EC, DIM], F32R, tag="w_sb")
                nc.vector.tensor_add(
                    out=w_sb.rearrange("p c d -> p (c d)"),
                    in0=w_ps.rearrange("p c d -> p (c d)"),
                    in1=dmat.rearrange("p c d -> p (c d)"),
                )

                # --- out = x @ W; two node tiles share one PSUM bank and are
                #     evicted with a single copy into the output half ---
                out_half = [
                    outp.tile([128, HT, DIM], F32, tag="o", name=f"o{i}_{hh}")
                    for hh in range(2)
                ]
                for tq in range(HT):
                    t0 = 2 * tq
                    opair = ps_b.tile([128, 2, DIM], F32, tag="bank", name=f"a{i}_{tq}")
                    for j, t in enumerate((t0, t0 + 1)):
                        for ecc in range(EC):
                            nc.tensor.matmul(
                                opair[:, j, :],
                                xt_sb[:, ecc, t * 128 : (t + 1) * 128],
                                w_sb[:, ecc, :],
                                start=(ecc == 0),
                                stop=(ecc == EC - 1),
                            )
                    dst = out_half[t0 // HT][:, t0 % HT : t0 % HT + 2, :]
                    if tq % 2 == 0:
                        nc.scalar.copy(out=dst, in_=opair)
                    else:
                        nc.vector.tensor_copy(out=dst, in_=opair)
                    if tq == HT // 2 - 1:
                        nc.gpsimd.dma_start(
                            out=x_hbm_out(out_d, i)[:, 0:HT, :], in_=out_half[0]
                        )
                nc.gpsimd.dma_start(
                    out=x_hbm_out(out_d, i)[:, HT:NT, :], in_=out_half[1]
                )

    nc.finalize()
    return nc


def x_hbm_out(out_d, i):
    return out_d[i].rearrange("(p t) d -> p t d", p=128)


def _host_prep(x, Q, K, alpha, beta):
    x = np.ascontiguousarray(np.asarray(x, dtype=np.float32))
    Q = np.asarray(Q, dtype=np.float32)
    K = np.asarray(K, dtype=np.float32)
    sa = (1.0 / (1.0 + np.exp(-np.asarray(alpha, dtype=np.float32)))).reshape(HEAD)
    sb = (1.0 / (1.0 + np.exp(-np.asarray(beta, dtype=np.float32)))).reshape(HEAD)

    scale_cols = np.repeat(sb / NN, HD).astype(np.float32)        # [256]
    qs = (Q * scale_cols[None, :]).astype(np.float32)
    qst = np.ascontiguousarray(qs.T)
    kw = np.ascontiguousarray(K)
    sa_cols = np.repeat(sa, HD).astype(np.float32)
    dmat = np.ascontiguousarray(np.diag(sa_cols).astype(np.float32))
    ident = np.ascontiguousarray(np.eye(128, dtype=np.float32))
    zed = np.zeros((128, EC * DIM), dtype=np.float32)

    x48 = x.reshape(BT, NN, DIM)
    in_maps = []
    for c in range(N_CORES):
        shard = np.ascontiguousarray(x48[c * BT_PER_CORE : (c + 1) * BT_PER_CORE])
        in_maps.append(
            {
                "x": shard,
                "kw": kw,
                "qst": qst,
                "dmat": dmat,
                "ident": ident,
                "zed": zed,
            }
        )
    return in_maps


def run(x, Q, K, alpha, beta, **spmd_kwargs):
    """Build, run on 8 cores, gather. Returns (out, BassKernelResults, nc)."""
    in_maps = _host_prep(x, Q, K, alpha, beta)
    nc = build_nc()
    res = run_bass_kernel_spmd(nc, in_maps, core_ids=list(range(N_CORES)), **spmd_kwargs)
    out48 = np.concatenate([res.results[c]["out"] for c in range(N_CORES)], axis=0)
    out = out48.reshape(B, T, NN, DIM).astype(np.float32, copy=False)
    return out, res, nc


def kernel(x, Q, K, alpha, beta):
    out, _, _ = run(x, Q, K, alpha, beta)
    return out
